# revision 48
# baseline (speedup 1.0000x reference)
"""Causal self-attention on 8 Trainium2 NeuronCores.

Reference (fp32):
    qkv = x @ W_qkv + b_qkv ; split q,k,v ; heads H=16, Dh=64
    scores = q @ k^T / sqrt(Dh), causal mask, softmax
    out = (attn @ v) re-merged ; y = out @ W_proj + b_proj

Sharding: tensor-parallel over heads x data-parallel over batch.
Core c (0..7) owns batch b = c//4 and head group g = c%4 (heads 4g..4g+3,
organized as pairs p=0,1 of two heads each). Each core computes
q^T,k^T,v for its 4 heads from x[b]^T, runs causal attention (scores in
transposed layout, exp without max-subtraction -- scores are O(5) so
fp32 exp is safe). The two heads' AV matmuls are col-tiled (M=64 at
array columns 0:64/64:128) and run CONCURRENTLY on the PE; the softmax
denominator is fp16-accumulated on the vector engine per k-tile and
partition-summed by one tiny matmul per block (an appended ones-column
would force M=65 and serialize the AV pair at ~51% array utilization).
As soon as a pair's output O^T block is normalized it is
AllGathered (fp16, 128KB) across the 4 cores of the batch; each core
then computes its own 256-row slice of y^T with a per-core
(row-permuted) W_proj column slice + bias. No reduction collective.

Scheduling (what got this from ~250us to ~230us):
- All bulk input loads are host-prepacked into [128, N] layouts and
  issued as a handful of big SWDGE (gpsimd-queue) DMAs: one
  InstDMACopy fans out over all 16 SDMA engines (~400 GB/s), while a
  HWDGE (sync/scalar-queue) DMA runs a single engine (~27 GB/s) and
  costs ~2us latency each - so only small/hidden traffic lives there.
- Fill work (next block's q/k chains, own block's k chains + v tiles,
  previous blocks' projections, oin staging) is chopped into ~2-matmul
  units and deadline-spread over every attention step, so the PE never
  idles behind the exp-gated AV matmul.
- The denominator repack + reciprocal chain runs in each pair's
  epilogue; the finish (recb broadcast, normalize, AllGather trigger)
  fires one pair-attention later with all inputs ready.
- oin staging DMAs never sit on the gpsimd queue between AllGather
  triggers (a completion wait there serializes the collectives).
- Tail: the last pair-0 finish triggers immediately after its
  attention (the mesh absorbs cross-core skew during pair-1's
  attention); the last pair ships as ONE AllGather (each collective
  pays a fixed multi-us mesh-sync cost), with its repack chain on the
  then-idle scalar ring and ag_in/staging/y on SWDGE.

Matmuls run fp16 (full PE speed, 8x finer mantissa than bf16); y is
returned fp16 (well within tolerance). End-to-end error vs the fp32
reference is ~5e-4 of max|y|. Run-to-run variance is ~+/-15us from
AllGather skew between cores.
"""

import numpy as np

import concourse.bacc as bacc
import concourse.mybir as mybir
import concourse.tile as tile
from concourse.bass_utils import run_bass_kernel_spmd

B = 2
T = 2048
C = 1024
H = 16
DH = 64
G = 4  # heads per core
N_CORES = 8
TQ = 512  # q-chunk width
NKT = T // 128  # k tiles per head
NJQ = T // TQ  # q chunks
NCK = C // 128  # contraction tiles over model dim
SCALE = 1.0 / np.sqrt(DH)
GROUPS = [[0, 1, 2, 3], [4, 5, 6, 7]]

F32 = mybir.dt.float32
FP16 = mybir.dt.float16
MM_DT = FP16
ATT_DT = FP16

_PROG = None


def _build_program():
    nc = bacc.Bacc(
        "TRN2", target_bir_lowering=False, debug=False, num_devices=N_CORES
    )
    # host-prepacked so each load is one big fully-contiguous DMA:
    # xt[p, j*NCK*TQ + k*TQ + c] = x^T[k*128+p, j*TQ+c]
    xt_d = nc.dram_tensor("xt", [128, NJQ * NCK * TQ], MM_DT, kind="ExternalInput").ap()
    # w*[p, k*W + m] = W[k*128+p, m]
    wq_d = nc.dram_tensor("wq", [128, NCK * G * DH], MM_DT, kind="ExternalInput").ap()
    wk_d = nc.dram_tensor("wk", [128, NCK * G * DH], MM_DT, kind="ExternalInput").ap()
    wv_d = nc.dram_tensor("wv", [128, NCK * G * DH], MM_DT, kind="ExternalInput").ap()
    # wp rows are pair-permuted on the host to match the AllGather's
    # rank-stacked row order: rows 0:512 belong to pair 0, 512:1024 pair 1
    wp_d = nc.dram_tensor("wp", [128, NCK * 2 * 128], MM_DT, kind="ExternalInput").ap()
    bq_d = nc.dram_tensor("bq", [G * DH, 1], F32, kind="ExternalInput").ap()
    bk_d = nc.dram_tensor("bk", [G * DH, 1], F32, kind="ExternalInput").ap()
    bv_d = nc.dram_tensor("bv", [1, G * DH], F32, kind="ExternalInput").ap()
    bp_d = nc.dram_tensor("bp", [2 * 128, 1], F32, kind="ExternalInput").ap()
    mask2_d = nc.dram_tensor("mask2", [128, 256], ATT_DT, kind="ExternalInput").ap()
    bc2_d = nc.dram_tensor("bc2", [2, 128], MM_DT, kind="ExternalInput").ap()
    onesb_d = nc.dram_tensor("onesb", [128, G], ATT_DT, kind="ExternalInput").ap()
    onesr_d = nc.dram_tensor("onesr", [1, 128], MM_DT, kind="ExternalInput").ap()
    ag_in = [
        [
            nc.dram_tensor(f"ag_in{j}_{p}", [128, TQ], ATT_DT).ap()
            for p in range(2)
        ]
        for j in range(NJQ)
    ]
    ag_out = [
        [
            nc.dram_tensor(f"ag_out{j}_{p}", [512, TQ], ATT_DT).ap()
            for p in range(2)
        ]
        for j in range(NJQ)
    ]
    ag_in3 = [
        nc.dram_tensor(f"ag_in3h{h}", [128, TQ // 2], ATT_DT).ap() for h in range(2)
    ]
    ag_out3 = [
        nc.dram_tensor(f"ag_out3h{h}", [512, TQ // 2], ATT_DT).ap()
        for h in range(2)
    ]
    y_d = nc.dram_tensor("y", [2 * 128, T], MM_DT, kind="ExternalOutput").ap()

    with tile.TileContext(nc) as tc:
        with (
            nc.allow_low_precision(reason="fp16 matmul pipeline by design"),
            tc.tile_pool(name="ll", bufs=1) as ll,
            tc.tile_pool(name="mm1", bufs=2, space="PSUM") as mm1,
            tc.tile_pool(name="spp", bufs=2, space="PSUM") as spp,
            tc.tile_pool(name="ovp", bufs=2, space="PSUM") as ovp,
            tc.tile_pool(name="esp", bufs=4) as esp,
            tc.tile_pool(name="accp", bufs=2) as accp,
            tc.tile_pool(name="dtp", bufs=6) as dtp,
            tc.tile_pool(name="rpp", bufs=4) as rpp,
            tc.tile_pool(name="oip", bufs=4) as oip,
            tc.tile_pool(name="yop", bufs=2) as yop,
        ):
            # ---- long-lived tiles -------------------------------------
            qT = [ll.tile([128, T], ATT_DT, tag=f"qT{p}", name=f"qT{p}") for p in range(2)]
            kT = [ll.tile([128, T], ATT_DT, tag=f"kT{p}", name=f"kT{p}") for p in range(2)]
            oT = [ll.tile([128, T], ATT_DT, tag=f"oT{p}", name=f"oT{p}") for p in range(2)]
            vaug = [ll.tile([128, G * 65], ATT_DT, tag=f"va{t}", name=f"va{t}") for t in range(NKT)]

            # prepacked SBUF parks: one tile per tensor, big contiguous DMAs
            WQW = G * DH  # 256 cols per k-tile
            xtall = ll.tile([128, NJQ * NCK * TQ], MM_DT, tag="xtall")
            wqall = ll.tile([128, NCK * WQW], MM_DT, tag="wqall")
            wkall = ll.tile([128, NCK * WQW], MM_DT, tag="wkall")
            wvall = ll.tile([128, NCK * WQW], MM_DT, tag="wvall")
            wpall = ll.tile([128, NCK * 2 * 128], MM_DT, tag="wpall")

            def xt_sl(k, j, lo, hi):
                base = j * NCK * TQ + k * TQ
                return xtall[:, base + lo : base + hi]

            JW = NCK * TQ  # one j-chunk of xt, 4096 cols
            # big loads all on gpsimd (SWDGE: one InstDMACopy fans out
            # over all 16 SDMA engines, ~400 GB/s). HWDGE (sync/scalar)
            # runs a single engine (~27 GB/s) - only tiny DMAs there.
            # earliest-needed first: first qk chain needs wq + xt(j0).
            nc.gpsimd.dma_start(out=wqall[:], in_=wq_d[:])
            nc.gpsimd.dma_start(
                out=xtall[:, 0 : JW // 2], in_=xt_d[:, 0 : JW // 2]
            )
            nc.gpsimd.dma_start(
                out=xtall[:, JW // 2 : JW], in_=xt_d[:, JW // 2 : JW]
            )
            nc.gpsimd.dma_start(out=wkall[:], in_=wk_d[:])
            # small constants (gpsimd too - HWDGE pays ~2us serial latency
            # per tiny DMA, SWDGE generation is ~100ns)
            bq_sb = [ll.tile([128, 1], F32, tag=f"bq{p}", name=f"bq{p}") for p in range(2)]
            bk_sb = [ll.tile([128, 1], F32, tag=f"bk{p}", name=f"bk{p}") for p in range(2)]
            for p in range(2):
                nc.gpsimd.dma_start(
                    out=bq_sb[p][:], in_=bq_d[p * 128 : (p + 1) * 128, :]
                )
                nc.gpsimd.dma_start(
                    out=bk_sb[p][:], in_=bk_d[p * 128 : (p + 1) * 128, :]
                )
            bv_sb = ll.tile([1, G * DH], F32, tag="bv")
            nc.gpsimd.dma_start(out=bv_sb[:], in_=bv_d[:])
            mask2 = ll.tile([128, 256], ATT_DT, tag="mask2")
            nc.gpsimd.dma_start(out=mask2[:], in_=mask2_d[:])
            bc2_sb = ll.tile([2, 128], MM_DT, tag="bc2")
            nc.gpsimd.dma_start(out=bc2_sb[:], in_=bc2_d[:])
            onesr_sb = ll.tile([1, 128], MM_DT, tag="onesr")
            nc.gpsimd.dma_start(out=onesr_sb[:], in_=onesr_d[:])
            onesb_sb = ll.tile([128, G], ATT_DT, tag="onesb")
            nc.gpsimd.dma_start(out=onesb_sb[:], in_=onesb_d[:])
            bp_sb = [ll.tile([128, 1], F32, tag=f"bp{i}", name=f"bp{i}") for i in range(2)]
            for i in range(2):
                nc.gpsimd.dma_start(
                    out=bp_sb[i][:], in_=bp_d[i * 128 : (i + 1) * 128, :]
                )
            nc.gpsimd.dma_start(out=wvall[:], in_=wv_d[:])
            # remaining xt chunks + wp (first needed by proj(0), late)
            nc.gpsimd.dma_start(out=xtall[:, JW : 2 * JW], in_=xt_d[:, JW : 2 * JW])
            nc.gpsimd.dma_start(
                out=xtall[:, 2 * JW : 3 * JW], in_=xt_d[:, 2 * JW : 3 * JW]
            )
            nc.gpsimd.dma_start(
                out=xtall[:, 3 * JW : 4 * JW], in_=xt_d[:, 3 * JW : 4 * JW]
            )
            nc.gpsimd.dma_start(out=wpall[:], in_=wp_d[:])

            def wq_sl(k, p):
                return wqall[:, k * WQW + p * 128 : k * WQW + (p + 1) * 128]

            def wk_sl(k, p):
                return wkall[:, k * WQW + p * 128 : k * WQW + (p + 1) * 128]

            def wv_sl(k):
                return wvall[:, k * WQW : (k + 1) * WQW]

            def wp_sl(i, et):
                base = i * 256 + et * 128
                return wpall[:, base : base + 128]

            # bv broadcast across partitions (via ones-row matmul);
            # emitted AFTER the first qk chains so its wait on the const
            # DMAs never blocks the PE queue head at startup
            bv_r = ll.tile([1, G * DH], MM_DT, tag="bvr")
            bvb_sb = ll.tile([128, G * DH], F32, tag="bvb")

            def emit_bvb():
                nc.vector.tensor_copy(out=bv_r[:], in_=bv_sb[:])
                bvb_ps = mm1.tile([128, G * DH], F32, tag="mm1", name="bvbps")
                nc.tensor.matmul(
                    bvb_ps[:], lhsT=onesr_sb[:], rhs=bv_r[:], start=True, stop=True
                )
                nc.vector.tensor_copy(out=bvb_sb[:], in_=bvb_ps[:])

            mask3 = mask2.rearrange("p (h c) -> p h c", c=128)

            # ---- phase A building blocks ------------------------------
            def emit_qk_chain(j, which, p):
                wsl, bsb, dst = (
                    (wq_sl, bq_sb, qT) if which == "q" else (wk_sl, bk_sb, kT)
                )
                ps = mm1.tile([128, TQ], F32, tag="mm1", name="qkps")
                for k in range(NCK):
                    nc.tensor.matmul(
                        ps[:],
                        lhsT=wsl(k, p),
                        rhs=xt_sl(k, j, 0, TQ),
                        start=(k == 0),
                        stop=(k == NCK - 1),
                    )
                nc.vector.tensor_scalar_add(
                    out=dst[p][:, j * TQ : (j + 1) * TQ],
                    in0=ps[:],
                    scalar1=bsb[p][:],
                )

            def emit_v_tile(t):
                ps = mm1.tile([128, G * DH], F32, tag="mm1", name="vps")
                tj, toff = t // 4, (t % 4) * 128
                for k in range(NCK):
                    nc.tensor.matmul(
                        ps[:],
                        lhsT=xt_sl(k, tj, toff, toff + 128),
                        rhs=wv_sl(k),
                        start=(k == 0),
                        stop=(k == NCK - 1),
                    )
                va = vaug[t].rearrange("p (h x) -> p h x", x=65)
                nc.vector.tensor_add(
                    out=va[:, :, 0:64],
                    in0=ps[:].rearrange("p (h x) -> p h x", x=64),
                    in1=bvb_sb[:].rearrange("p (h x) -> p h x", x=64),
                )


            # ---- fine-grained fill units (~2 matmuls each) so every
            # attention step can absorb independent PE work while the
            # exp chain runs ------------------------------------------
            def qk_chain_units(j, which, p):
                wsl, bsb, dst = (
                    (wq_sl, bq_sb, qT) if which == "q" else (wk_sl, bk_sb, kT)
                )
                state = {}

                def mk(i0):
                    def u():
                        if "ps" not in state:
                            state["ps"] = mm1.tile(
                                [128, TQ], F32, tag="mm1", name="qkps"
                            )
                        ps = state["ps"]
                        for k in range(i0, i0 + 2):
                            nc.tensor.matmul(
                                ps[:],
                                lhsT=wsl(k, p),
                                rhs=xt_sl(k, j, 0, TQ),
                                start=(k == 0),
                                stop=(k == NCK - 1),
                            )
                        if i0 + 2 == NCK:
                            nc.vector.tensor_scalar_add(
                                out=dst[p][:, j * TQ : (j + 1) * TQ],
                                in0=ps[:],
                                scalar1=bsb[p][:],
                            )

                    return u

                return [mk(i) for i in range(0, NCK, 2)]

            def v_tile_units(t):
                state = {}
                tj, toff = t // 4, (t % 4) * 128

                def mk(i0):
                    def u():
                        if "ps" not in state:
                            state["ps"] = mm1.tile(
                                [128, G * DH], F32, tag="mm1", name="vps"
                            )
                        ps = state["ps"]
                        for k in range(i0, i0 + 2):
                            nc.tensor.matmul(
                                ps[:],
                                lhsT=xt_sl(k, tj, toff, toff + 128),
                                rhs=wv_sl(k),
                                start=(k == 0),
                                stop=(k == NCK - 1),
                            )
                        if i0 + 2 == NCK:
                            va = vaug[t].rearrange("p (h x) -> p h x", x=65)
                            nc.vector.tensor_add(
                                out=va[:, :, 0:64],
                                in0=ps[:].rearrange("p (h x) -> p h x", x=64),
                                in1=bvb_sb[:].rearrange("p (h x) -> p h x", x=64),
                            )


                    return u

                return [mk(i) for i in range(0, NCK, 2)]

            def proj_units(jq, et):
                state = {}

                def mk(idx):
                    def u():
                        if "ps" not in state:
                            state["ps"] = mm1.tile(
                                [128, TQ], F32, tag="mm1", name="pps"
                            )
                        ps = state["ps"]
                        for i in range(2 * idx, 2 * idx + 2):
                            p, ko = divmod(i, 4)
                            oin = oin_map[(jq, p)]
                            nc.tensor.matmul(
                                ps[:],
                                lhsT=wp_sl(i, et),
                                rhs=oin[:, ko * TQ : (ko + 1) * TQ],
                                start=(i == 0),
                                stop=(i == 7),
                            )
                        if idx == 3:
                            yo = yop.tile([128, TQ], MM_DT, tag="yo", name="yo")
                            nc.vector.tensor_scalar_add(
                                out=yo[:], in0=ps[:], scalar1=bp_sb[et][:]
                            )
                            nc.sync.dma_start(
                                out=y_d[
                                    et * 128 : (et + 1) * 128,
                                    jq * TQ : (jq + 1) * TQ,
                                ],
                                in_=yo[:],
                            )

                    return u

                return [mk(i) for i in range(4)]

            # ---- attention --------------------------------------------
            den_map = {}
            oin_map = {}

            def emit_stage(jq, p):
                # stage the gathered O^T rows for the projection (gpsimd
                # queue; by now the AllGather has had a full pair-attention
                # to complete, so this does not stall the queue head)
                oin = oip.tile([128, 4 * TQ], MM_DT, tag="oin", name="oin")
                for ko in range(4):
                    nc.gpsimd.dma_start(
                        out=oin[:, ko * TQ : (ko + 1) * TQ],
                        in_=ag_out[jq][p][ko * 128 : (ko + 1) * 128, :],
                    )
                oin_map[(jq, p)] = oin

            def emit_attention_pair(jq, p, fill):
                # S/exp/mask/V pipeline for pair p over q block jq. `fill`
                # is a list of closures (independent PE work) spread between
                # the attention steps so the PE pipe never drains while the
                # scalar engine works through the exp stream.
                kmax = 4 * jq + 4
                nf = len(fill)
                fi = 0
                hor = max(1, kmax - 1)
                # both heads' AV matmuls are col-tiled (M=64 at array
                # columns 0:64 / 64:128) and run CONCURRENTLY - the ones
                # denominator column is gone (it forced M=65 = full-array
                # serialization); the denominator is DVE-accumulated below
                ov2 = ovp.tile([128, TQ], F32, tag="ov", name="ov2")
                acc = accp.tile([128, 2 * TQ], ATT_DT, tag="acc", name="acc")
                a3 = acc.rearrange("p (h q) -> p h q", q=TQ)

                def emit_v(kt, qlo, es2):
                    va = vaug[kt].rearrange("p (h x) -> p h x", x=65)
                    for half in range(2):
                        nc.tensor.matmul(
                            ov2[64 * half : 64 * half + 64, qlo:TQ],
                            lhsT=va[:, 2 * p + half, 0:64],
                            rhs=es2[:, half * TQ + qlo : (half + 1) * TQ],
                            start=(kt == 0),
                            stop=(kt == kmax - 1),
                        )

                prev = None
                for kt in range(kmax):
                    # diagonal tiles only contribute to q >= k: narrow the
                    # S-matmul/exp/mask/V to the valid q-range
                    d = kt - 4 * jq
                    qlo = 128 * d if d >= 0 else 0
                    sps2 = spp.tile([128, 2 * TQ], F32, tag="s", name="sps2")
                    for half in range(2):
                        r = 64 * half
                        nc.tensor.matmul(
                            sps2[:, half * TQ + qlo : (half + 1) * TQ],
                            lhsT=kT[p][r : r + 64, kt * 128 : (kt + 1) * 128],
                            rhs=qT[p][r : r + 64, jq * TQ + qlo : (jq + 1) * TQ],
                            start=True,
                            stop=True,
                        )
                    es2 = esp.tile([128, 2 * TQ], ATT_DT, tag="es", name="es2")
                    s3 = sps2.rearrange("p (h q) -> p h q", q=TQ)
                    e3 = es2.rearrange("p (h q) -> p h q", q=TQ)
                    nc.scalar.activation(
                        out=e3[:, :, qlo:TQ],
                        in_=s3[:, :, qlo:TQ],
                        func=mybir.ActivationFunctionType.Exp,
                        scale=SCALE,
                    )
                    if d >= 0:
                        # causal mask is only non-trivial on the 128-column
                        # band that straddles the diagonal
                        nc.vector.tensor_mul(
                            out=e3[:, :, qlo : qlo + 128],
                            in0=e3[:, :, qlo : qlo + 128],
                            in1=mask3[:],
                        )
                    # denominator partials: per-lane sum of the exp tiles
                    if kt == 0:
                        nc.vector.tensor_copy(out=a3[:, :, :], in_=e3[:, :, :])
                    else:
                        nc.vector.tensor_add(
                            out=a3[:, :, qlo:TQ],
                            in0=a3[:, :, qlo:TQ],
                            in1=e3[:, :, qlo:TQ],
                        )
                    if prev is not None:
                        emit_v(*prev)
                    prev = (kt, qlo, es2)
                    # fire fill units due this step (deadline-spread so
                    # producers land before their consuming steps)
                    target = min(nf, -(-((kt + 1) * nf) // hor))
                    while fi < target:
                        fill[fi]()
                        fi += 1
                emit_v(*prev)
                while fi < nf:
                    fill[fi]()
                    fi += 1
                # epilogue: move unnormalized O out, pack denominators into
                # a lane-parallel [128, 8] layout for the finish's reciprocal
                dq = nc.scalar if (jq, p) == (NJQ - 1, 1) else nc.sync
                nc.vector.tensor_copy(
                    out=oT[p][:, jq * TQ : (jq + 1) * TQ], in_=ov2[:, :]
                )
                den2 = dtp.tile([128, 8], F32, tag="den2", name="den2")
                for half in range(2):
                    dps = mm1.tile([1, TQ], F32, tag="mm1", name="dps")
                    nc.tensor.matmul(
                        dps[:],
                        lhsT=onesb_sb[:, 0:1],
                        rhs=acc[:, half * TQ : (half + 1) * TQ],
                        start=True,
                        stop=True,
                    )
                    dt_t = dtp.tile([1, TQ], F32, tag="dt", name="dt")
                    nc.vector.tensor_copy(out=dt_t[:], in_=dps[:])
                    dq.dma_start(
                        out=den2[:, 4 * half : 4 * half + 4], in_=dt_t[:]
                    )
                rec2 = dtp.tile([128, 8], MM_DT, tag="rec2", name="rec2")
                nc.vector.reciprocal(out=rec2[:], in_=den2[:])
                rp_t = rpp.tile([2, TQ], MM_DT, tag="rp", name="rp")
                for half in range(2):
                    dq.dma_start(
                        out=rp_t[half : half + 1, :],
                        in_=rec2[:, 4 * half : 4 * half + 4],
                    )
                den_map[(jq, p)] = rp_t

            def emit_pair_finish(jq, p, split=False, last=False):
                # normalize pair p of block jq and launch its AllGather;
                # fired as a fill one pair-attention later. The whole
                # reciprocal/repack chain ran in the epilogue, so the
                # recb matmul's inputs are ready when this fires.
                rp_t = den_map[(jq, p)]
                recb = mm1.tile([128, TQ], F32, tag="mm1", name="recb")
                nc.tensor.matmul(
                    recb[:], lhsT=bc2_sb[:], rhs=rp_t[:], start=True, stop=True
                )
                dst = oT[p][:, jq * TQ : (jq + 1) * TQ]
                nc.vector.tensor_mul(out=dst, in0=dst, in1=recb[:])
                if split:
                    hw = TQ // 2
                    for h in range(2):
                        nc.sync.dma_start(
                            out=ag_in3[h][:],
                            in_=oT[p][
                                :, jq * TQ + h * hw : jq * TQ + (h + 1) * hw
                            ],
                        )
                        nc.gpsimd.collective_compute(
                            "AllGather",
                            mybir.AluOpType.bypass,
                            ins=[ag_in3[h][:]],
                            outs=[ag_out3[h][:]],
                            replica_groups=GROUPS,
                        )
                else:
                    aq = nc.gpsimd if last else nc.sync
                    aq.dma_start(out=ag_in[jq][p][:], in_=dst)
                    nc.gpsimd.collective_compute(
                        "AllGather",
                        mybir.AluOpType.bypass,
                        ins=[ag_in[jq][p][:]],
                        outs=[ag_out[jq][p][:]],
                        replica_groups=GROUPS,
                    )

            def emit_stage3(jq, p, h):
                hw = TQ // 2
                oin = oip.tile([128, 4 * hw], MM_DT, tag="oin3", name="oin3")
                for ko in range(4):
                    nc.gpsimd.dma_start(
                        out=oin[:, ko * hw : (ko + 1) * hw],
                        in_=ag_out3[h][ko * 128 : (ko + 1) * 128, :],
                    )
                oin_map[(jq, p, h)] = oin

            def emit_proj_half(jq, et, h):
                hw = TQ // 2
                ps = mm1.tile([128, hw], F32, tag="mm1", name="pps")
                oin0 = oin_map[(jq, 0)]
                oin1 = oin_map[(jq, 1, h)]
                first = True
                for p in range(2):
                    for ko in range(4):
                        rhs = (
                            oin0[:, ko * TQ + h * hw : ko * TQ + (h + 1) * hw]
                            if p == 0
                            else oin1[:, ko * hw : (ko + 1) * hw]
                        )
                        nc.tensor.matmul(
                            ps[:],
                            lhsT=wp_sl(4 * p + ko, et),
                            rhs=rhs,
                            start=first,
                            stop=(p == 1 and ko == 3),
                        )
                        first = False
                yo = yop.tile([128, hw], MM_DT, tag="yo", name="yo")
                nc.vector.tensor_scalar_add(
                    out=yo[:], in0=ps[:], scalar1=bp_sb[et][:]
                )
                nc.sync.dma_start(
                    out=y_d[
                        et * 128 : (et + 1) * 128,
                        jq * TQ + h * hw : jq * TQ + (h + 1) * hw,
                    ],
                    in_=yo[:],
                )

            # ---- output projection (one 128-row slice of y^T) ---------
            def emit_proj(jq, et):
                ps = mm1.tile([128, TQ], F32, tag="mm1", name="pps")
                first = True
                for p in range(2):
                    oin = oin_map[(jq, p)]
                    for ko in range(4):
                        nc.tensor.matmul(
                            ps[:],
                            lhsT=wp_sl(4 * p + ko, et),
                            rhs=oin[:, ko * TQ : (ko + 1) * TQ],
                            start=first,
                            stop=(p == 1 and ko == 3),
                        )
                        first = False
                yo = yop.tile([128, TQ], MM_DT, tag="yo", name="yo")
                nc.vector.tensor_scalar_add(
                    out=yo[:], in0=ps[:], scalar1=bp_sb[et][:]
                )
                nc.gpsimd.dma_start(
                    out=y_d[et * 128 : (et + 1) * 128, jq * TQ : (jq + 1) * TQ],
                    in_=yo[:],
                )

            # ---- main emission ----------------------------------------
            # Block 0's q/k chains first (PE unblocks on wq+xt j0), then
            # the bvb broadcast + v tiles. Later blocks' phase A, the
            # projections, stagings and finishes are chopped into ~2-MM
            # units and spread over every attention step: q chains of
            # block j+1 during att(j,0/1); k chains + v tiles of block j
            # during att(j,*) itself (only needed at its diagonal steps);
            # proj(j) during att(j+1,1) onward.
            for p in range(2):
                emit_qk_chain(0, "q", p)
            for p in range(2):
                emit_qk_chain(0, "k", p)
            emit_bvb()
            for t in range(4):
                emit_v_tile(t)

            def qu(j):
                return qk_chain_units(j, "q", 0) + qk_chain_units(j, "q", 1)

            def ku(j, p):
                return qk_chain_units(j, "k", p)

            def vu(t):
                return v_tile_units(t)

            def pu(jq):
                return proj_units(jq, 0) + proj_units(jq, 1)

            fills = {
                (0, 0): qu(1),
                (0, 1): ku(1, 0) + ku(1, 1),
                (1, 0): [lambda: emit_stage(0, 0)]
                + vu(4) + vu(5) + vu(6) + vu(7),
                (1, 1): [lambda: emit_stage(0, 1)] + qu(2),
                (2, 0): [lambda: emit_stage(1, 0)]
                + ku(2, 0) + vu(8) + vu(9) + vu(10) + vu(11) + ku(2, 1),
                (2, 1): [lambda: emit_stage(1, 1)] + qu(3) + pu(0),
                (3, 0): [lambda: emit_stage(2, 0)]
                + ku(3, 0) + vu(12) + vu(13) + vu(14) + vu(15) + ku(3, 1),
                (3, 1): [lambda: emit_stage(2, 1)]
                + pu(1) + pu(2)
                + [lambda: emit_stage(3, 0)],
            }
            for jq in range(NJQ):
                half0 = list(fills[(jq, 0)])
                if jq >= 1:
                    half0.insert(0, lambda j=jq - 1: emit_pair_finish(j, 1))
                half1 = list(fills[(jq, 1)])
                if jq < NJQ - 1:
                    half1.insert(0, lambda j=jq: emit_pair_finish(j, 0))
                emit_attention_pair(jq, 0, half0)
                if jq == NJQ - 1:
                    # last block: trigger pair-0's AllGather right at the
                    # end of its attention so the mesh has the whole of
                    # pair-1's attention to absorb cross-core skew
                    emit_pair_finish(jq, 0)
                emit_attention_pair(jq, 1, half1)
            # tail: ONE whole-block AllGather for the last pair (each
            # collective pays a fixed mesh-sync cost that dwarfs the
            # transfer, so two halves lose), everything on the idle
            # gpsimd/SWDGE path, then stage + both projections
            emit_pair_finish(NJQ - 1, 1, last=True)
            emit_stage(NJQ - 1, 1)
            emit_proj(NJQ - 1, 0)
            emit_proj(NJQ - 1, 1)

    nc.compile()
    return nc


def _get_program():
    global _PROG
    if _PROG is None:
        _PROG = _build_program()
    return _PROG


def kernel(x, W_qkv, b_qkv, W_proj, b_proj):
    x = np.asarray(x, dtype=np.float32)
    W_qkv = np.asarray(W_qkv, dtype=np.float32)
    b_qkv = np.asarray(b_qkv, dtype=np.float32)
    W_proj = np.asarray(W_proj, dtype=np.float32)
    b_proj = np.asarray(b_proj, dtype=np.float32)

    nc = _get_program()

    mm_np = np.float16
    att_np = np.float16
    kl = np.arange(128)[:, None]
    jj = np.arange(128)[None, :]
    band = (jj >= kl).astype(att_np)
    mask2_host = np.concatenate([band, band], axis=1)
    bc2_host = np.zeros((2, 128), dtype=mm_np)
    bc2_host[0, 0:64] = 1.0
    bc2_host[1, 64:128] = 1.0
    onesb_host = np.ones((128, G), dtype=att_np)
    onesr_host = np.ones((1, 128), dtype=mm_np)

    # W_proj rows permuted to the AllGather's rank-stacked pair order:
    # for pair p the gathered rows are [g0:(h=2p, h=2p+1), g1:(...), ...]
    perm = np.concatenate(
        [
            np.arange(64 * (4 * g + 2 * p + e), 64 * (4 * g + 2 * p + e) + 64)
            for p in range(2)
            for g in range(4)
            for e in range(2)
        ]
    )
    wp_perm = W_proj[perm, :]

    def wpack(w):
        # [C, M] -> [128, NCK*M]: row k*128+p, col m  ->  [p, k*M+m]
        m = w.shape[1]
        return np.ascontiguousarray(
            w.reshape(NCK, 128, m).transpose(1, 0, 2).reshape(128, NCK * m)
        ).astype(mm_np)

    def xpack(xb):
        # x[b] [T, C] -> x^T [C, T] -> [128, NJQ*NCK*TQ]:
        # row k*128+p, col j*TQ+c  ->  [p, j*NCK*TQ + k*TQ + c]
        xt = xb.T.reshape(NCK, 128, NJQ, TQ)
        return np.ascontiguousarray(
            xt.transpose(1, 2, 0, 3).reshape(128, NJQ * NCK * TQ)
        ).astype(mm_np)

    xts = [xpack(x[b]) for b in range(B)]
    in_maps = []
    for c in range(N_CORES):
        b, g = divmod(c, 4)
        cs = slice(g * G * DH, (g + 1) * G * DH)
        in_maps.append(
            {
                "xt": xts[b],
                "wq": wpack(W_qkv[:, cs]),
                "wk": wpack(W_qkv[:, C:][:, cs]),
                "wv": wpack(W_qkv[:, 2 * C :][:, cs]),
                "wp": wpack(wp_perm[:, cs]),
                "bq": np.ascontiguousarray(b_qkv[cs]).reshape(-1, 1),
                "bk": np.ascontiguousarray(b_qkv[C:][cs]).reshape(-1, 1),
                "bv": np.ascontiguousarray(b_qkv[2 * C :][cs]).reshape(1, -1),
                "bp": np.ascontiguousarray(b_proj[cs]).reshape(-1, 1),
                "mask2": mask2_host,
                "bc2": bc2_host,
                "onesb": onesb_host,
                "onesr": onesr_host,
            }
        )

    global _last_in_maps
    _last_in_maps = in_maps
    res = run_bass_kernel_spmd(nc, in_maps, list(range(N_CORES)))

    y = np.empty((B, T, C), dtype=np.float32)
    for b in range(B):
        yT = np.concatenate(
            [res.results[4 * b + r]["y"] for r in range(4)], axis=0
        )
        y[b] = yT.T.astype(np.float32)
    return y



# revision 49
# speedup vs baseline: 1.0073x; 1.0073x over previous
"""Causal self-attention on 8 Trainium2 NeuronCores.

Reference (fp32):
    qkv = x @ W_qkv + b_qkv ; split q,k,v ; heads H=16, Dh=64
    scores = q @ k^T / sqrt(Dh), causal mask, softmax
    out = (attn @ v) re-merged ; y = out @ W_proj + b_proj

Sharding: tensor-parallel over heads x data-parallel over batch.
Core c (0..7) owns batch b = c//4 and head group g = c%4 (heads 4g..4g+3,
organized as pairs p=0,1 of two heads each). Each core computes
q^T,k^T,v for its 4 heads from x[b]^T, runs causal attention (scores in
transposed layout, exp without max-subtraction -- scores are O(5) so
fp32 exp is safe). The two heads' AV matmuls are col-tiled (M=64 at
array columns 0:64/64:128) and run CONCURRENTLY on the PE; the softmax
denominator is fp16-accumulated on the vector engine per k-tile and
partition-summed by one tiny matmul per block (an appended ones-column
would force M=65 and serialize the AV pair at ~51% array utilization).
As soon as a pair's output O^T block is normalized it is
AllGathered (fp16, 128KB) across the 4 cores of the batch; each core
then computes its own 256-row slice of y^T with a per-core
(row-permuted) W_proj column slice + bias. No reduction collective.

Scheduling (what got this from ~250us to ~230us):
- All bulk input loads are host-prepacked into [128, N] layouts and
  issued as a handful of big SWDGE (gpsimd-queue) DMAs: one
  InstDMACopy fans out over all 16 SDMA engines (~400 GB/s), while a
  HWDGE (sync/scalar-queue) DMA runs a single engine (~27 GB/s) and
  costs ~2us latency each - so only small/hidden traffic lives there.
- Fill work (next block's q/k chains, own block's k chains + v tiles,
  previous blocks' projections, oin staging) is chopped into ~2-matmul
  units and deadline-spread over every attention step, so the PE never
  idles behind the exp-gated AV matmul.
- The denominator repack + reciprocal chain runs in each pair's
  epilogue; the finish (recb broadcast, normalize, AllGather trigger)
  fires one pair-attention later with all inputs ready.
- oin staging DMAs never sit on the gpsimd queue between AllGather
  triggers (a completion wait there serializes the collectives).
- Tail: the last pair-0 finish triggers immediately after its
  attention (the mesh absorbs cross-core skew during pair-1's
  attention); the last pair ships as ONE AllGather (each collective
  pays a fixed multi-us mesh-sync cost), with its repack chain on the
  then-idle scalar ring and ag_in/staging/y on SWDGE.

Matmuls run fp16 (full PE speed, 8x finer mantissa than bf16); y is
returned fp16 (well within tolerance). End-to-end error vs the fp32
reference is ~5e-4 of max|y|. Run-to-run variance is ~+/-15us from
AllGather skew between cores.
"""

import numpy as np

import concourse.bacc as bacc
import concourse.mybir as mybir
import concourse.tile as tile
from concourse.bass_utils import run_bass_kernel_spmd

B = 2
T = 2048
C = 1024
H = 16
DH = 64
G = 4  # heads per core
N_CORES = 8
TQ = 512  # q-chunk width
NKT = T // 128  # k tiles per head
NJQ = T // TQ  # q chunks
NCK = C // 128  # contraction tiles over model dim
SCALE = 1.0 / np.sqrt(DH)
GROUPS = [[0, 1, 2, 3], [4, 5, 6, 7]]

F32 = mybir.dt.float32
FP16 = mybir.dt.float16
MM_DT = FP16
ATT_DT = FP16

_PROG = None


def _build_program():
    nc = bacc.Bacc(
        "TRN2", target_bir_lowering=False, debug=False, num_devices=N_CORES
    )
    # host-prepacked so each load is one big fully-contiguous DMA:
    # xt[p, j*NCK*TQ + k*TQ + c] = x^T[k*128+p, j*TQ+c]
    xt_d = nc.dram_tensor("xt", [128, NJQ * NCK * TQ], MM_DT, kind="ExternalInput").ap()
    # w*[p, k*W + m] = W[k*128+p, m]
    wq_d = nc.dram_tensor("wq", [128, NCK * G * DH], MM_DT, kind="ExternalInput").ap()
    wk_d = nc.dram_tensor("wk", [128, NCK * G * DH], MM_DT, kind="ExternalInput").ap()
    wv_d = nc.dram_tensor("wv", [128, NCK * G * DH], MM_DT, kind="ExternalInput").ap()
    # wp rows are pair-permuted on the host to match the AllGather's
    # rank-stacked row order: rows 0:512 belong to pair 0, 512:1024 pair 1
    wp_d = nc.dram_tensor("wp", [128, NCK * 2 * 128], MM_DT, kind="ExternalInput").ap()
    bq_d = nc.dram_tensor("bq", [G * DH, 1], F32, kind="ExternalInput").ap()
    bk_d = nc.dram_tensor("bk", [G * DH, 1], F32, kind="ExternalInput").ap()
    bv_d = nc.dram_tensor("bv", [1, G * DH], F32, kind="ExternalInput").ap()
    bp_d = nc.dram_tensor("bp", [2 * 128, 1], F32, kind="ExternalInput").ap()
    mask2_d = nc.dram_tensor("mask2", [128, 256], ATT_DT, kind="ExternalInput").ap()
    bc2_d = nc.dram_tensor("bc2", [2, 128], MM_DT, kind="ExternalInput").ap()
    onesb_d = nc.dram_tensor("onesb", [128, G], ATT_DT, kind="ExternalInput").ap()
    onesr_d = nc.dram_tensor("onesr", [1, 128], MM_DT, kind="ExternalInput").ap()
    ag_in = [
        [
            nc.dram_tensor(f"ag_in{j}_{p}", [128, TQ], ATT_DT).ap()
            for p in range(2)
        ]
        for j in range(NJQ)
    ]
    ag_out = [
        [
            nc.dram_tensor(f"ag_out{j}_{p}", [512, TQ], ATT_DT).ap()
            for p in range(2)
        ]
        for j in range(NJQ)
    ]
    ag_in3 = [
        nc.dram_tensor(f"ag_in3h{h}", [128, TQ // 2], ATT_DT).ap() for h in range(2)
    ]
    ag_out3 = [
        nc.dram_tensor(f"ag_out3h{h}", [512, TQ // 2], ATT_DT).ap()
        for h in range(2)
    ]
    y_d = nc.dram_tensor("y", [2 * 128, T], MM_DT, kind="ExternalOutput").ap()

    with tile.TileContext(nc) as tc:
        with (
            nc.allow_low_precision(reason="fp16 matmul pipeline by design"),
            tc.tile_pool(name="ll", bufs=1) as ll,
            tc.tile_pool(name="mm1", bufs=2, space="PSUM") as mm1,
            tc.tile_pool(name="spp", bufs=2, space="PSUM") as spp,
            tc.tile_pool(name="ovp", bufs=2, space="PSUM") as ovp,
            tc.tile_pool(name="esp", bufs=4) as esp,
            tc.tile_pool(name="accp", bufs=2) as accp,
            tc.tile_pool(name="dtp", bufs=6) as dtp,
            tc.tile_pool(name="rpp", bufs=4) as rpp,
            tc.tile_pool(name="oip", bufs=4) as oip,
            tc.tile_pool(name="yop", bufs=2) as yop,
        ):
            # ---- long-lived tiles -------------------------------------
            qT = [ll.tile([128, T], ATT_DT, tag=f"qT{p}", name=f"qT{p}") for p in range(2)]
            kT = [ll.tile([128, T], ATT_DT, tag=f"kT{p}", name=f"kT{p}") for p in range(2)]
            oT = [ll.tile([128, T], ATT_DT, tag=f"oT{p}", name=f"oT{p}") for p in range(2)]
            vaug = [ll.tile([128, G * 65], ATT_DT, tag=f"va{t}", name=f"va{t}") for t in range(NKT)]

            # prepacked SBUF parks: one tile per tensor, big contiguous DMAs
            WQW = G * DH  # 256 cols per k-tile
            xtall = ll.tile([128, NJQ * NCK * TQ], MM_DT, tag="xtall")
            wqall = ll.tile([128, NCK * WQW], MM_DT, tag="wqall")
            wkall = ll.tile([128, NCK * WQW], MM_DT, tag="wkall")
            wvall = ll.tile([128, NCK * WQW], MM_DT, tag="wvall")
            wpall = ll.tile([128, NCK * 2 * 128], MM_DT, tag="wpall")

            def xt_sl(k, j, lo, hi):
                base = j * NCK * TQ + k * TQ
                return xtall[:, base + lo : base + hi]

            JW = NCK * TQ  # one j-chunk of xt, 4096 cols
            # big loads all on gpsimd (SWDGE: one InstDMACopy fans out
            # over all 16 SDMA engines, ~400 GB/s). HWDGE (sync/scalar)
            # runs a single engine (~27 GB/s) - only tiny DMAs there.
            # earliest-needed first: first qk chain needs wq + xt(j0).
            mask2 = ll.tile([128, 256], ATT_DT, tag="mask2")
            nc.gpsimd.dma_start(out=mask2[:], in_=mask2_d[:])
            nc.gpsimd.dma_start(out=wqall[:], in_=wq_d[:])
            nc.gpsimd.dma_start(
                out=xtall[:, 0 : JW // 2], in_=xt_d[:, 0 : JW // 2]
            )
            nc.gpsimd.dma_start(
                out=xtall[:, JW // 2 : JW], in_=xt_d[:, JW // 2 : JW]
            )
            nc.gpsimd.dma_start(out=wkall[:], in_=wk_d[:])
            # small constants (gpsimd too - HWDGE pays ~2us serial latency
            # per tiny DMA, SWDGE generation is ~100ns)
            bq_sb = [ll.tile([128, 1], F32, tag=f"bq{p}", name=f"bq{p}") for p in range(2)]
            bk_sb = [ll.tile([128, 1], F32, tag=f"bk{p}", name=f"bk{p}") for p in range(2)]
            for p in range(2):
                nc.gpsimd.dma_start(
                    out=bq_sb[p][:], in_=bq_d[p * 128 : (p + 1) * 128, :]
                )
                nc.gpsimd.dma_start(
                    out=bk_sb[p][:], in_=bk_d[p * 128 : (p + 1) * 128, :]
                )
            bv_sb = ll.tile([1, G * DH], F32, tag="bv")
            nc.gpsimd.dma_start(out=bv_sb[:], in_=bv_d[:])
            bc2_sb = ll.tile([2, 128], MM_DT, tag="bc2")
            nc.gpsimd.dma_start(out=bc2_sb[:], in_=bc2_d[:])
            onesr_sb = ll.tile([1, 128], MM_DT, tag="onesr")
            nc.gpsimd.dma_start(out=onesr_sb[:], in_=onesr_d[:])
            onesb_sb = ll.tile([128, G], ATT_DT, tag="onesb")
            nc.gpsimd.dma_start(out=onesb_sb[:], in_=onesb_d[:])
            bp_sb = [ll.tile([128, 1], F32, tag=f"bp{i}", name=f"bp{i}") for i in range(2)]
            for i in range(2):
                nc.gpsimd.dma_start(
                    out=bp_sb[i][:], in_=bp_d[i * 128 : (i + 1) * 128, :]
                )
            nc.gpsimd.dma_start(out=wvall[:], in_=wv_d[:])
            # remaining xt chunks + wp (first needed by proj(0), late)
            nc.gpsimd.dma_start(out=xtall[:, JW : 2 * JW], in_=xt_d[:, JW : 2 * JW])
            nc.gpsimd.dma_start(
                out=xtall[:, 2 * JW : 3 * JW], in_=xt_d[:, 2 * JW : 3 * JW]
            )
            nc.gpsimd.dma_start(
                out=xtall[:, 3 * JW : 4 * JW], in_=xt_d[:, 3 * JW : 4 * JW]
            )
            nc.gpsimd.dma_start(out=wpall[:], in_=wp_d[:])

            def wq_sl(k, p):
                return wqall[:, k * WQW + p * 128 : k * WQW + (p + 1) * 128]

            def wk_sl(k, p):
                return wkall[:, k * WQW + p * 128 : k * WQW + (p + 1) * 128]

            def wv_sl(k):
                return wvall[:, k * WQW : (k + 1) * WQW]

            def wp_sl(i, et):
                base = i * 256 + et * 128
                return wpall[:, base : base + 128]

            # bv broadcast across partitions (via ones-row matmul);
            # emitted AFTER the first qk chains so its wait on the const
            # DMAs never blocks the PE queue head at startup
            bv_r = ll.tile([1, G * DH], MM_DT, tag="bvr")
            bvb_sb = ll.tile([128, G * DH], F32, tag="bvb")

            def emit_bvb():
                nc.vector.tensor_copy(out=bv_r[:], in_=bv_sb[:])
                bvb_ps = mm1.tile([128, G * DH], F32, tag="mm1", name="bvbps")
                nc.tensor.matmul(
                    bvb_ps[:], lhsT=onesr_sb[:], rhs=bv_r[:], start=True, stop=True
                )
                nc.vector.tensor_copy(out=bvb_sb[:], in_=bvb_ps[:])

            mask3 = mask2.rearrange("p (h c) -> p h c", c=128)

            # ---- phase A building blocks ------------------------------
            def emit_qk_chain(j, which, p):
                wsl, bsb, dst = (
                    (wq_sl, bq_sb, qT) if which == "q" else (wk_sl, bk_sb, kT)
                )
                ps = mm1.tile([128, TQ], F32, tag="mm1", name="qkps")
                for k in range(NCK):
                    nc.tensor.matmul(
                        ps[:],
                        lhsT=wsl(k, p),
                        rhs=xt_sl(k, j, 0, TQ),
                        start=(k == 0),
                        stop=(k == NCK - 1),
                    )
                nc.vector.tensor_scalar_add(
                    out=dst[p][:, j * TQ : (j + 1) * TQ],
                    in0=ps[:],
                    scalar1=bsb[p][:],
                )

            def emit_v_tile(t):
                ps = mm1.tile([128, G * DH], F32, tag="mm1", name="vps")
                tj, toff = t // 4, (t % 4) * 128
                for k in range(NCK):
                    nc.tensor.matmul(
                        ps[:],
                        lhsT=xt_sl(k, tj, toff, toff + 128),
                        rhs=wv_sl(k),
                        start=(k == 0),
                        stop=(k == NCK - 1),
                    )
                va = vaug[t].rearrange("p (h x) -> p h x", x=65)
                nc.vector.tensor_add(
                    out=va[:, :, 0:64],
                    in0=ps[:].rearrange("p (h x) -> p h x", x=64),
                    in1=bvb_sb[:].rearrange("p (h x) -> p h x", x=64),
                )


            # ---- fine-grained fill units (~2 matmuls each) so every
            # attention step can absorb independent PE work while the
            # exp chain runs ------------------------------------------
            def qk_chain_units(j, which, p):
                wsl, bsb, dst = (
                    (wq_sl, bq_sb, qT) if which == "q" else (wk_sl, bk_sb, kT)
                )
                state = {}

                def mk(i0):
                    def u():
                        if "ps" not in state:
                            state["ps"] = mm1.tile(
                                [128, TQ], F32, tag="mm1", name="qkps"
                            )
                        ps = state["ps"]
                        for k in range(i0, i0 + 2):
                            nc.tensor.matmul(
                                ps[:],
                                lhsT=wsl(k, p),
                                rhs=xt_sl(k, j, 0, TQ),
                                start=(k == 0),
                                stop=(k == NCK - 1),
                            )
                        if i0 + 2 == NCK:
                            nc.vector.tensor_scalar_add(
                                out=dst[p][:, j * TQ : (j + 1) * TQ],
                                in0=ps[:],
                                scalar1=bsb[p][:],
                            )

                    return u

                return [mk(i) for i in range(0, NCK, 2)]

            def v_tile_units(t):
                state = {}
                tj, toff = t // 4, (t % 4) * 128

                def mk(i0):
                    def u():
                        if "ps" not in state:
                            state["ps"] = mm1.tile(
                                [128, G * DH], F32, tag="mm1", name="vps"
                            )
                        ps = state["ps"]
                        for k in range(i0, i0 + 2):
                            nc.tensor.matmul(
                                ps[:],
                                lhsT=xt_sl(k, tj, toff, toff + 128),
                                rhs=wv_sl(k),
                                start=(k == 0),
                                stop=(k == NCK - 1),
                            )
                        if i0 + 2 == NCK:
                            va = vaug[t].rearrange("p (h x) -> p h x", x=65)
                            nc.vector.tensor_add(
                                out=va[:, :, 0:64],
                                in0=ps[:].rearrange("p (h x) -> p h x", x=64),
                                in1=bvb_sb[:].rearrange("p (h x) -> p h x", x=64),
                            )


                    return u

                return [mk(i) for i in range(0, NCK, 2)]

            def proj_units(jq, et):
                state = {}

                def mk(idx):
                    def u():
                        if "ps" not in state:
                            state["ps"] = mm1.tile(
                                [128, TQ], F32, tag="mm1", name="pps"
                            )
                        ps = state["ps"]
                        for i in range(2 * idx, 2 * idx + 2):
                            p, ko = divmod(i, 4)
                            oin = oin_map[(jq, p)]
                            nc.tensor.matmul(
                                ps[:],
                                lhsT=wp_sl(i, et),
                                rhs=oin[:, ko * TQ : (ko + 1) * TQ],
                                start=(i == 0),
                                stop=(i == 7),
                            )
                        if idx == 3:
                            yo = yop.tile([128, TQ], MM_DT, tag="yo", name="yo")
                            nc.vector.tensor_scalar_add(
                                out=yo[:], in0=ps[:], scalar1=bp_sb[et][:]
                            )
                            nc.sync.dma_start(
                                out=y_d[
                                    et * 128 : (et + 1) * 128,
                                    jq * TQ : (jq + 1) * TQ,
                                ],
                                in_=yo[:],
                            )

                    return u

                return [mk(i) for i in range(4)]

            # ---- attention --------------------------------------------
            den_map = {}
            oin_map = {}

            def emit_stage(jq, p):
                # stage the gathered O^T rows for the projection (gpsimd
                # queue; by now the AllGather has had a full pair-attention
                # to complete, so this does not stall the queue head)
                oin = oip.tile([128, 4 * TQ], MM_DT, tag="oin", name="oin")
                for ko in range(4):
                    nc.gpsimd.dma_start(
                        out=oin[:, ko * TQ : (ko + 1) * TQ],
                        in_=ag_out[jq][p][ko * 128 : (ko + 1) * 128, :],
                    )
                oin_map[(jq, p)] = oin

            def emit_attention_pair(jq, p, fill):
                # S/exp/mask/V pipeline for pair p over q block jq. `fill`
                # is a list of closures (independent PE work) spread between
                # the attention steps so the PE pipe never drains while the
                # scalar engine works through the exp stream.
                kmax = 4 * jq + 4
                nf = len(fill)
                fi = 0
                hor = max(1, kmax - 1)
                # both heads' AV matmuls are col-tiled (M=64 at array
                # columns 0:64 / 64:128) and run CONCURRENTLY - the ones
                # denominator column is gone (it forced M=65 = full-array
                # serialization); the denominator is DVE-accumulated below
                ov2 = ovp.tile([128, TQ], F32, tag="ov", name="ov2")
                acc = accp.tile([128, 2 * TQ], ATT_DT, tag="acc", name="acc")
                a3 = acc.rearrange("p (h q) -> p h q", q=TQ)

                def emit_v(kt, qlo, es2):
                    va = vaug[kt].rearrange("p (h x) -> p h x", x=65)
                    for half in range(2):
                        nc.tensor.matmul(
                            ov2[64 * half : 64 * half + 64, qlo:TQ],
                            lhsT=va[:, 2 * p + half, 0:64],
                            rhs=es2[:, half * TQ + qlo : (half + 1) * TQ],
                            start=(kt == 0),
                            stop=(kt == kmax - 1),
                        )

                prev = None
                for kt in range(kmax):
                    # diagonal tiles only contribute to q >= k: narrow the
                    # S-matmul/exp/mask/V to the valid q-range
                    d = kt - 4 * jq
                    qlo = 128 * d if d >= 0 else 0
                    sps2 = spp.tile([128, 2 * TQ], F32, tag="s", name="sps2")
                    for half in range(2):
                        r = 64 * half
                        nc.tensor.matmul(
                            sps2[:, half * TQ + qlo : (half + 1) * TQ],
                            lhsT=kT[p][r : r + 64, kt * 128 : (kt + 1) * 128],
                            rhs=qT[p][r : r + 64, jq * TQ + qlo : (jq + 1) * TQ],
                            start=True,
                            stop=True,
                        )
                    es2 = esp.tile([128, 2 * TQ], ATT_DT, tag="es", name="es2")
                    s3 = sps2.rearrange("p (h q) -> p h q", q=TQ)
                    e3 = es2.rearrange("p (h q) -> p h q", q=TQ)
                    nc.scalar.activation(
                        out=e3[:, :, qlo:TQ],
                        in_=s3[:, :, qlo:TQ],
                        func=mybir.ActivationFunctionType.Exp,
                        scale=SCALE,
                    )
                    if d >= 0:
                        # causal mask is only non-trivial on the 128-column
                        # band that straddles the diagonal
                        nc.vector.tensor_mul(
                            out=e3[:, :, qlo : qlo + 128],
                            in0=e3[:, :, qlo : qlo + 128],
                            in1=mask3[:],
                        )
                    # denominator partials: per-lane sum of the exp tiles
                    if kt == 0:
                        nc.vector.tensor_copy(out=a3[:, :, :], in_=e3[:, :, :])
                    else:
                        nc.vector.tensor_add(
                            out=a3[:, :, qlo:TQ],
                            in0=a3[:, :, qlo:TQ],
                            in1=e3[:, :, qlo:TQ],
                        )
                    if prev is not None:
                        emit_v(*prev)
                    prev = (kt, qlo, es2)
                    # fire fill units due this step (deadline-spread so
                    # producers land before their consuming steps)
                    target = min(nf, -(-((kt + 1) * nf) // hor))
                    while fi < target:
                        fill[fi]()
                        fi += 1
                emit_v(*prev)
                while fi < nf:
                    fill[fi]()
                    fi += 1
                # epilogue: move unnormalized O out, pack denominators into
                # a lane-parallel [128, 8] layout for the finish's reciprocal
                dq = nc.scalar if (jq, p) == (NJQ - 1, 1) else nc.sync
                nc.vector.tensor_copy(
                    out=oT[p][:, jq * TQ : (jq + 1) * TQ], in_=ov2[:, :]
                )
                den2 = dtp.tile([128, 8], F32, tag="den2", name="den2")
                for half in range(2):
                    dps = mm1.tile([1, TQ], F32, tag="mm1", name="dps")
                    nc.tensor.matmul(
                        dps[:],
                        lhsT=onesb_sb[:, 0:1],
                        rhs=acc[:, half * TQ : (half + 1) * TQ],
                        start=True,
                        stop=True,
                    )
                    dt_t = dtp.tile([1, TQ], F32, tag="dt", name="dt")
                    nc.vector.tensor_copy(out=dt_t[:], in_=dps[:])
                    dq.dma_start(
                        out=den2[:, 4 * half : 4 * half + 4], in_=dt_t[:]
                    )
                rec2 = dtp.tile([128, 8], MM_DT, tag="rec2", name="rec2")
                nc.vector.reciprocal(out=rec2[:], in_=den2[:])
                rp_t = rpp.tile([2, TQ], MM_DT, tag="rp", name="rp")
                for half in range(2):
                    dq.dma_start(
                        out=rp_t[half : half + 1, :],
                        in_=rec2[:, 4 * half : 4 * half + 4],
                    )
                den_map[(jq, p)] = rp_t

            def emit_pair_finish(jq, p, split=False, last=False):
                # normalize pair p of block jq and launch its AllGather;
                # fired as a fill one pair-attention later. The whole
                # reciprocal/repack chain ran in the epilogue, so the
                # recb matmul's inputs are ready when this fires.
                rp_t = den_map[(jq, p)]
                recb = mm1.tile([128, TQ], F32, tag="mm1", name="recb")
                nc.tensor.matmul(
                    recb[:], lhsT=bc2_sb[:], rhs=rp_t[:], start=True, stop=True
                )
                dst = oT[p][:, jq * TQ : (jq + 1) * TQ]
                nc.vector.tensor_mul(out=dst, in0=dst, in1=recb[:])
                if split:
                    hw = TQ // 2
                    for h in range(2):
                        nc.sync.dma_start(
                            out=ag_in3[h][:],
                            in_=oT[p][
                                :, jq * TQ + h * hw : jq * TQ + (h + 1) * hw
                            ],
                        )
                        nc.gpsimd.collective_compute(
                            "AllGather",
                            mybir.AluOpType.bypass,
                            ins=[ag_in3[h][:]],
                            outs=[ag_out3[h][:]],
                            replica_groups=GROUPS,
                        )
                else:
                    aq = nc.gpsimd if last else nc.sync
                    aq.dma_start(out=ag_in[jq][p][:], in_=dst)
                    nc.gpsimd.collective_compute(
                        "AllGather",
                        mybir.AluOpType.bypass,
                        ins=[ag_in[jq][p][:]],
                        outs=[ag_out[jq][p][:]],
                        replica_groups=GROUPS,
                    )

            def emit_stage3(jq, p, h):
                hw = TQ // 2
                oin = oip.tile([128, 4 * hw], MM_DT, tag="oin3", name="oin3")
                for ko in range(4):
                    nc.gpsimd.dma_start(
                        out=oin[:, ko * hw : (ko + 1) * hw],
                        in_=ag_out3[h][ko * 128 : (ko + 1) * 128, :],
                    )
                oin_map[(jq, p, h)] = oin

            def emit_proj_half(jq, et, h):
                hw = TQ // 2
                ps = mm1.tile([128, hw], F32, tag="mm1", name="pps")
                oin0 = oin_map[(jq, 0)]
                oin1 = oin_map[(jq, 1, h)]
                first = True
                for p in range(2):
                    for ko in range(4):
                        rhs = (
                            oin0[:, ko * TQ + h * hw : ko * TQ + (h + 1) * hw]
                            if p == 0
                            else oin1[:, ko * hw : (ko + 1) * hw]
                        )
                        nc.tensor.matmul(
                            ps[:],
                            lhsT=wp_sl(4 * p + ko, et),
                            rhs=rhs,
                            start=first,
                            stop=(p == 1 and ko == 3),
                        )
                        first = False
                yo = yop.tile([128, hw], MM_DT, tag="yo", name="yo")
                nc.vector.tensor_scalar_add(
                    out=yo[:], in0=ps[:], scalar1=bp_sb[et][:]
                )
                nc.sync.dma_start(
                    out=y_d[
                        et * 128 : (et + 1) * 128,
                        jq * TQ + h * hw : jq * TQ + (h + 1) * hw,
                    ],
                    in_=yo[:],
                )

            # ---- output projection (one 128-row slice of y^T) ---------
            def emit_proj(jq, et):
                ps = mm1.tile([128, TQ], F32, tag="mm1", name="pps")
                first = True
                for p in range(2):
                    oin = oin_map[(jq, p)]
                    for ko in range(4):
                        nc.tensor.matmul(
                            ps[:],
                            lhsT=wp_sl(4 * p + ko, et),
                            rhs=oin[:, ko * TQ : (ko + 1) * TQ],
                            start=first,
                            stop=(p == 1 and ko == 3),
                        )
                        first = False
                yo = yop.tile([128, TQ], MM_DT, tag="yo", name="yo")
                nc.vector.tensor_scalar_add(
                    out=yo[:], in0=ps[:], scalar1=bp_sb[et][:]
                )
                nc.gpsimd.dma_start(
                    out=y_d[et * 128 : (et + 1) * 128, jq * TQ : (jq + 1) * TQ],
                    in_=yo[:],
                )

            # HAM warm-up: the PE would idle ~6-13us waiting for the
            # weight DMAs and then pay the cold 1.2GHz clock for its first
            # ~16 real matmuls. Throwaway matmuls on the first-arriving
            # constant keep the HAM activity window busy; results go to a
            # scratch PSUM tile that is never read.
            hamw = mm1.tile([128, 256], F32, tag="mm1", name="hamwarm")
            for i in range(24):
                nc.tensor.matmul(
                    hamw[:],
                    lhsT=mask2[:, 0:128],
                    rhs=mask2[:, 0:256],
                    start=(i == 0),
                    stop=(i == 23),
                )

            # ---- main emission ----------------------------------------
            # Block 0's q/k chains first (PE unblocks on wq+xt j0), then
            # the bvb broadcast + v tiles. Later blocks' phase A, the
            # projections, stagings and finishes are chopped into ~2-MM
            # units and spread over every attention step: q chains of
            # block j+1 during att(j,0/1); k chains + v tiles of block j
            # during att(j,*) itself (only needed at its diagonal steps);
            # proj(j) during att(j+1,1) onward.
            for p in range(2):
                emit_qk_chain(0, "q", p)
            for p in range(2):
                emit_qk_chain(0, "k", p)
            emit_bvb()
            for t in range(4):
                emit_v_tile(t)

            def qu(j):
                return qk_chain_units(j, "q", 0) + qk_chain_units(j, "q", 1)

            def ku(j, p):
                return qk_chain_units(j, "k", p)

            def vu(t):
                return v_tile_units(t)

            def pu(jq):
                return proj_units(jq, 0) + proj_units(jq, 1)

            fills = {
                (0, 0): qu(1),
                (0, 1): ku(1, 0) + ku(1, 1),
                (1, 0): [lambda: emit_stage(0, 0)]
                + vu(4) + vu(5) + vu(6) + vu(7),
                (1, 1): [lambda: emit_stage(0, 1)] + qu(2),
                (2, 0): [lambda: emit_stage(1, 0)]
                + ku(2, 0) + vu(8) + vu(9) + vu(10) + vu(11) + ku(2, 1),
                (2, 1): [lambda: emit_stage(1, 1)] + qu(3) + pu(0),
                (3, 0): [lambda: emit_stage(2, 0)]
                + ku(3, 0) + vu(12) + vu(13) + vu(14) + vu(15) + ku(3, 1),
                (3, 1): [lambda: emit_stage(2, 1)]
                + pu(1) + pu(2)
                + [lambda: emit_stage(3, 0)],
            }
            for jq in range(NJQ):
                half0 = list(fills[(jq, 0)])
                if jq >= 1:
                    half0.insert(0, lambda j=jq - 1: emit_pair_finish(j, 1))
                half1 = list(fills[(jq, 1)])
                if jq < NJQ - 1:
                    half1.insert(0, lambda j=jq: emit_pair_finish(j, 0))
                emit_attention_pair(jq, 0, half0)
                if jq == NJQ - 1:
                    # last block: trigger pair-0's AllGather right at the
                    # end of its attention so the mesh has the whole of
                    # pair-1's attention to absorb cross-core skew
                    emit_pair_finish(jq, 0)
                emit_attention_pair(jq, 1, half1)
            # tail: ONE whole-block AllGather for the last pair (each
            # collective pays a fixed mesh-sync cost that dwarfs the
            # transfer, so two halves lose), everything on the idle
            # gpsimd/SWDGE path, then stage + both projections
            emit_pair_finish(NJQ - 1, 1, last=True)
            emit_stage(NJQ - 1, 1)
            emit_proj(NJQ - 1, 0)
            emit_proj(NJQ - 1, 1)

    nc.compile()
    return nc


def _get_program():
    global _PROG
    if _PROG is None:
        _PROG = _build_program()
    return _PROG


def kernel(x, W_qkv, b_qkv, W_proj, b_proj):
    x = np.asarray(x, dtype=np.float32)
    W_qkv = np.asarray(W_qkv, dtype=np.float32)
    b_qkv = np.asarray(b_qkv, dtype=np.float32)
    W_proj = np.asarray(W_proj, dtype=np.float32)
    b_proj = np.asarray(b_proj, dtype=np.float32)

    nc = _get_program()

    mm_np = np.float16
    att_np = np.float16
    kl = np.arange(128)[:, None]
    jj = np.arange(128)[None, :]
    band = (jj >= kl).astype(att_np)
    mask2_host = np.concatenate([band, band], axis=1)
    bc2_host = np.zeros((2, 128), dtype=mm_np)
    bc2_host[0, 0:64] = 1.0
    bc2_host[1, 64:128] = 1.0
    onesb_host = np.ones((128, G), dtype=att_np)
    onesr_host = np.ones((1, 128), dtype=mm_np)

    # W_proj rows permuted to the AllGather's rank-stacked pair order:
    # for pair p the gathered rows are [g0:(h=2p, h=2p+1), g1:(...), ...]
    perm = np.concatenate(
        [
            np.arange(64 * (4 * g + 2 * p + e), 64 * (4 * g + 2 * p + e) + 64)
            for p in range(2)
            for g in range(4)
            for e in range(2)
        ]
    )
    wp_perm = W_proj[perm, :]

    def wpack(w):
        # [C, M] -> [128, NCK*M]: row k*128+p, col m  ->  [p, k*M+m]
        m = w.shape[1]
        return np.ascontiguousarray(
            w.reshape(NCK, 128, m).transpose(1, 0, 2).reshape(128, NCK * m)
        ).astype(mm_np)

    def xpack(xb):
        # x[b] [T, C] -> x^T [C, T] -> [128, NJQ*NCK*TQ]:
        # row k*128+p, col j*TQ+c  ->  [p, j*NCK*TQ + k*TQ + c]
        xt = xb.T.reshape(NCK, 128, NJQ, TQ)
        return np.ascontiguousarray(
            xt.transpose(1, 2, 0, 3).reshape(128, NJQ * NCK * TQ)
        ).astype(mm_np)

    xts = [xpack(x[b]) for b in range(B)]
    in_maps = []
    for c in range(N_CORES):
        b, g = divmod(c, 4)
        cs = slice(g * G * DH, (g + 1) * G * DH)
        in_maps.append(
            {
                "xt": xts[b],
                "wq": wpack(W_qkv[:, cs]),
                "wk": wpack(W_qkv[:, C:][:, cs]),
                "wv": wpack(W_qkv[:, 2 * C :][:, cs]),
                "wp": wpack(wp_perm[:, cs]),
                "bq": np.ascontiguousarray(b_qkv[cs]).reshape(-1, 1),
                "bk": np.ascontiguousarray(b_qkv[C:][cs]).reshape(-1, 1),
                "bv": np.ascontiguousarray(b_qkv[2 * C :][cs]).reshape(1, -1),
                "bp": np.ascontiguousarray(b_proj[cs]).reshape(-1, 1),
                "mask2": mask2_host,
                "bc2": bc2_host,
                "onesb": onesb_host,
                "onesr": onesr_host,
            }
        )

    global _last_in_maps
    _last_in_maps = in_maps
    res = run_bass_kernel_spmd(nc, in_maps, list(range(N_CORES)))

    y = np.empty((B, T, C), dtype=np.float32)
    for b in range(B):
        yT = np.concatenate(
            [res.results[4 * b + r]["y"] for r in range(4)], axis=0
        )
        y[b] = yT.T.astype(np.float32)
    return y

